# revision 31
# baseline (speedup 1.0000x reference)
"""Dense CRF forward (5 mean-field iterations, exact dense bilateral kernel)
on 8 Trainium2 NeuronCores via Bass/Tile.

Sharding: core c -> (batch n = c//4, group-rank g = c%4). Each core owns 1024
of the 4096 pixels of one batch element and keeps resident in SBUF the
[4096 x 1024] column-block W of 4*K in fp16 (half the f32 footprint), built
once by an fp16 hi/lo-split feature matmul + scalar exp writing fp16.

qbf per iteration: W-moving matmuls (512-column streams keep the PE array
duty cycle high enough for the HAM clock gate to hold 2.4 GHz -- a
W-stationary variant with 42-col moving operands measured 2.6x slower purely
from half-clock + per-instruction overheads). q is carried as an fp16 hi/lo
pair: stationary qh tile [128,21] accumulates into PSUM rows 0-20, and (for
the core's own block, whose residual is available locally) ql into rows
32-52 (32 for tile_position legality). Hi+lo are combined and transposed to
pixel-major by the vector engine (copy + add + 32x128 DVE transposes), then
unary + separable fp16 conv + softmax epilogue as usual. Host-side sim of
this quantization (remote blocks qh-only, own block exact): 8.1e-3 max err
vs the fp32 reference (tolerance 2e-2; the previous all-exact variant
measured 2.7e-3 on HW vs 4.9e-3 sim).

Exchange per iteration: each core p2p-broadcasts its new qh block (fp16,
336 B/partition -- half the f32 baseline's payload) to its 3 same-die peers
on slots 0-2 (bit-2 slots are reserved for cross-die routing). XOR span
layout (block g^s at span s) keeps the program SPMD-uniform; per-span remote
semaphores let each span's matmuls start as soon as its message lands.

Scheduling discipline: the Tile scheduler validates against a sim whose p2p
semaphores are pre-charged, so nothing stops it hoisting a p2p wait above
the instructions that feed this core's own sends -- which deadlocks every
core on hardware. All keep-warm fillers and waits are therefore chained by
explicit deps onto the local matmul stream.

PSUM discipline: start=True marks the full 2 KiB bank pending-zero on the
instruction's own partitions; accumulation groups on disjoint partition rows
are independent, and every matmul output stays inside one bank.
"""
import os
import sys

for _p in ("/opt/trn_rl_repo", "/root/.axon_site/_ro/trn_rl_repo"):
    if os.path.isdir(_p) and _p not in sys.path:
        sys.path.insert(0, _p)

import numpy as np
import concourse.bass as bass  # noqa: E402
import concourse.bass_interp as _bi  # noqa: E402
import concourse.tile as tile  # noqa: E402
from concourse import mybir, bacc  # noqa: E402
from concourse.bass_utils import run_bass_kernel_spmd  # noqa: E402
from concourse.tile_rust import add_dep_helper  # noqa: E402

# The Tile scheduler's single-core virtual sim cannot model peer-to-peer
# semaphore increments (remote_dma remote_sem updates land on other cores),
# so any wait on them would falsely deadlock the SCHEDULING pass. Pre-charge
# those semaphores in the scheduler's virtual state only -- the compiled
# program is unchanged and still waits at runtime.
_SIM_PRECHARGE: dict[int, tuple[str, int]] = {}
_orig_coresim_simulate = _bi.CoreSim.simulate


def _simulate_with_precharge(self):
    for sem_id, (sem_name, val) in _SIM_PRECHARGE.items():
        upd = mybir.SyncUpdate(
            sync_type="semaphore", id=sem_id, ant_name=sem_name,
            update_mode="sem-add-imm", update_value=val)
        self.update_semaphore(upd)
    return _orig_coresim_simulate(self)


_bi.CoreSim.simulate = _simulate_with_precharge

F32 = mybir.dt.float32
FP16 = mybir.dt.float16
EXP = mybir.ActivationFunctionType.Exp
AX = mybir.AxisListType
ALU = mybir.AluOpType

N, C, H, W_IMG = 2, 21, 64, 64
P = H * W_IMG            # 4096 pixels
NB = 4                   # blocks (cores) per batch element
PB = P // NB             # 1024 pixels per block
T = P // 128             # 32 q-tiles of 128 pixels
PC = PB // 128           # 8 p-chunks of 128 pixels per block
NUM_ITER = int(os.environ.get("KNUM_ITER", "5"))
COMPAT_BF, COMPAT_SPATIAL = 4.0, 2.0
KD = 18                  # split-feature contraction dims (fp16, 11-bit mantissa)
QCOLS = T * C            # 672 columns of one full q block copy
LQ = 32                  # psum partition base of the lo accumulation rows

RSEM_PER_ROUND = 2       # per-span sem: one 1-dest broadcast -> +2
LSEM_PER_ROUND = 48      # 3 broadcasts x local_sem += 16

TRACE = False
LAST_EXEC_NS = None
LAST_RESULTS = None

_CACHED_NC = None


def _build_program():
    nc = bacc.Bacc("TRN2", target_bir_lowering=False, debug=False, num_devices=8)

    fA_d = nc.dram_tensor("fa", [KD, P], FP16, kind="ExternalInput")
    fB_d = nc.dram_tensor("fb", [KD, PB], FP16, kind="ExternalInput")
    u_d = nc.dram_tensor("u_blk", [128, PC * C], F32, kind="ExternalInput")
    qhl0_d = nc.dram_tensor("qhl0i", [128, T * 53], FP16, kind="ExternalInput")
    q0yl_d = nc.dram_tensor("q0yl", [64, H * C], FP16, kind="ExternalInput")
    a2_d = nc.dram_tensor("a2mat", [64, 64], FP16, kind="ExternalInput")
    ay_d = nc.dram_tensor("ay", [64, 16], FP16, kind="ExternalInput")
    out_d = nc.dram_tensor("out_blk", [128, PC * C], F32, kind="ExternalOutput")
    id_d = nc.inline_tensor(np.eye(128, dtype=np.float32), name="ident_np")

    rsems = [nc.alloc_semaphore(f"p2p_recv{m}") for m in (1, 2, 3)]
    lsem = nc.alloc_semaphore("p2p_sent")
    _SIM_PRECHARGE.clear()
    for rs in rsems:
        _SIM_PRECHARGE[rs.num] = (rs.name, RSEM_PER_ROUND * (NUM_ITER - 1))
    _SIM_PRECHARGE[lsem.num] = (lsem.name, LSEM_PER_ROUND * (NUM_ITER - 1))

    with tile.TileContext(nc) as tc:
        with (
            tc.tile_pool(name="const", bufs=1) as cpool,
            tc.tile_pool(name="wpool", bufs=1) as wpool,
            tc.tile_pool(name="qpool", bufs=1) as qpool,
            tc.tile_pool(name="work", bufs=1) as work,
            tc.tile_pool(name="ps_build", bufs=3, space="PSUM") as ps_build,
            tc.tile_pool(name="ps_qbf", bufs=2, space="PSUM") as ps_qbf,
            tc.tile_pool(name="ps_conv", bufs=1, space="PSUM") as ps_conv,
        ):
            # Dummy collective: a NEFF containing a collective gets its 8
            # cores gang-launched by the runtime; without one, per-core
            # dispatch skew reaches ~10ms and every p2p wait eats it. The
            # result is never read -- this exists purely for launch sync.
            sync_in = nc.dram_tensor("syncin", [2, 256], F32, kind="Internal")
            sync_out = nc.dram_tensor("syncout", [8, 256], F32, kind="Internal")
            nc.gpsimd.collective_compute(
                "AllGather", ALU.bypass,
                replica_groups=[[0, 1, 2, 3], [4, 5, 6, 7]],
                ins=[sync_in[:]], outs=[sync_out[:]])

            # ---- constants / inputs to SBUF ----
            fa = cpool.tile([KD, P], FP16, tag="fa")
            fb = cpool.tile([KD, PB], FP16, tag="fb")
            u_t = cpool.tile([128, PC * C], F32, tag="u_t")
            a2 = cpool.tile([64, 64], FP16, tag="a2")
            ay = cpool.tile([64, 16], FP16, tag="ay")
            ident = cpool.tile([128, 128], F32, tag="ident")
            identh = cpool.tile([128, 512], FP16, tag="identh")
            nc.sync.dma_start(fa[:], fA_d[:])
            nc.sync.dma_start(fb[:], fB_d[:])
            nc.sync.dma_start(u_t[:], u_d[:])
            nc.scalar.dma_start(a2[:], a2_d[:])
            nc.scalar.dma_start(ay[:], ay_d[:])
            nc.scalar.dma_start(ident[:], id_d[:])
            nc.vector.memset(identh[:], 0)

            # ---- persistent q tiles (fp16), double-buffered by parity ----
            # qh: full block copy, span s at cols [168s, 168s+168).
            # ql: lo residuals; only span 0 (own block) is refreshed after
            # iteration 0 -- remote lo columns are used at iteration 0 only.
            qh_a = qpool.tile([128, QCOLS], FP16, tag="qh_a")
            qh_b = qpool.tile([128, QCOLS], FP16, tag="qh_b")
            ql_a = qpool.tile([128, QCOLS], FP16, tag="ql_a")
            ql_b = qpool.tile([128, QCOLS], FP16, tag="ql_b")
            qh = [qh_a, qh_b]
            ql = [ql_a, ql_b]
            qhl0i = qpool.tile([128, T * 53], FP16, tag="qhl0i")
            qyl = qpool.tile([64, H * C], FP16, tag="qyl")
            nc.sync.dma_start(qhl0i[:], qhl0_d[:])
            nc.sync.dma_start(qyl[:], q0yl_d[:])

            w_sb = wpool.tile([128, T * PB], FP16, tag="wsb")

            # ---- working tiles ----
            t0 = work.tile([128, PC * C], F32, tag="t0")
            e_t = work.tile([128, PC * C], F32, tag="e_t")
            uc = work.tile([128, PC * C], F32, tag="uc")
            ssum = work.tile([128, PC], F32, tag="ssum")
            rsum = work.tile([128, PC], F32, tag="rsum")
            qf = work.tile([128, PC * C], F32, tag="qf")
            s_cmb = work.tile([C, PB], F32, tag="s_cmb")
            s_t2 = work.tile([64, C * 16], FP16, tag="s_t2")
            qout = work.tile([128, PC * C], F32, tag="qout")

            def filler(pqt):
                """Keep-warm matmul into the unused partitions 64-127 of
                the live qbf psum tile (same handle -> tracked; disjoint
                rows -> the qbf accumulation is untouched)."""
                return nc.tensor.matmul(pqt[64:128, 0:512], identh[:, :64],
                                        identh[:], start=True, stop=True)

            def mm_hi(pq, buf, j, h, start, stop):
                """psum rows 0-20, cols [512h,512h+512) += qh_j^T-stationary
                over W tile j's pixel half h."""
                return nc.tensor.matmul(
                    pq[0:C, h * 512:(h + 1) * 512],
                    buf[:, j * C:(j + 1) * C],
                    w_sb[:, j * PB + h * 512: j * PB + (h + 1) * 512],
                    start=start, stop=stop)

            def mm_lo(pq, buf, j, h, start, stop):
                """Same for the lo residual, accumulating at rows 32-52."""
                return nc.tensor.matmul(
                    pq[LQ:LQ + C, h * 512:(h + 1) * 512],
                    buf[:, j * C:(j + 1) * C],
                    w_sb[:, j * PB + h * 512: j * PB + (h + 1) * 512],
                    start=start, stop=stop)

            def emit_conv(deps):
                """T2 stage: per-channel Toeplitz matmul over all 64 rows."""
                qyl_v = qyl[:].rearrange("p (x c) -> p c x", c=C)
                for ci in range(C):
                    mm = nc.tensor.matmul(
                        pt2[0:64, ci * 16:(ci + 1) * 16],
                        qyl_v[:, ci, :], ay[:],
                        start=True, stop=True)
                    for dep in deps:
                        add_dep_helper(mm.ins, dep.ins,
                                       reason="conv T2 waits gather/span")
                return mm

            def emit_conv_tail():
                """pt2 -> s_t2 -> T3 -> uc = u + conv."""
                nc.vector.tensor_copy(s_t2[:], pt2[0:64, :C * 16])
                p3 = ps_conv.tile([128, 512], F32, tag="pt2")
                mm = nc.tensor.matmul(p3[0:64, :C * 16], a2[:], s_t2[:],
                                      start=True, stop=True)
                t3v = p3[0:64, :C * 16].rearrange("p (c pc ylo) -> p ylo pc c",
                                                  pc=PC, ylo=2)
                for ylo in range(2):
                    dst = uc[ylo * 64:(ylo + 1) * 64, :].rearrange(
                        "p (pc c) -> p pc c", c=C)
                    src = u_t[ylo * 64:(ylo + 1) * 64, :].rearrange(
                        "p (pc c) -> p pc c", c=C)
                    nc.vector.tensor_tensor(dst, src, t3v[:, ylo], op=ALU.add)
                return mm

            def emit_epilogue(pq, it, h):
                """Combine hi+lo, transpose to pixel-major, softmax, and (for
                non-final iterations) produce the next qh/ql fp16 pair."""
                hc = slice(h * 512, (h + 1) * 512)
                # hi + lo -> s_cmb rows 0-20
                nc.vector.tensor_copy(s_cmb[0:C, hc], pq[0:C, hc])
                nc.vector.tensor_tensor(s_cmb[0:C, hc], s_cmb[0:C, hc],
                                        pq[LQ:LQ + C, hc], op=ALU.add)
                # PE transposes [21,128] -> [128,21] into the pbt psum tile
                for pc in range(4 * h, 4 * h + 4):
                    nc.tensor.transpose(
                        pbt[:, pc * C:(pc + 1) * C],
                        s_cmb[0:C, pc * 128:(pc + 1) * 128], ident[:C, :C])
                cs = slice(h * 4 * C, (h + 1) * 4 * C)
                nc.vector.tensor_tensor(t0[:, cs], pbt[:, cs], uc[:, cs],
                                        op=ALU.add)
                nc.scalar.activation(e_t[:, cs], t0[:, cs], EXP,
                                     bias=0.0, scale=1.0)
                nc.vector.tensor_reduce(
                    ssum[:, 4 * h:4 * h + 4],
                    e_t[:, cs].rearrange("p (pc c) -> p pc c", c=C),
                    axis=AX.X, op=ALU.add)
                nc.vector.reciprocal(rsum[:, 4 * h:4 * h + 4],
                                     ssum[:, 4 * h:4 * h + 4])
                dst_q = qout if it == NUM_ITER - 1 else qf
                for pc in range(4 * h, 4 * h + 4):
                    nc.vector.tensor_scalar_mul(
                        dst_q[:, pc * C:(pc + 1) * C],
                        e_t[:, pc * C:(pc + 1) * C], rsum[:, pc:pc + 1])
                if it < NUM_ITER - 1:
                    hs = slice(h * 4 * C, (h + 1) * 4 * C)
                    nh, nl = qh[(it + 1) % 2], ql[(it + 1) % 2]
                    # qh (span 0) = fp16 round of the new q
                    nc.vector.tensor_copy(nh[:, hs], qf[:, hs])
                    # ql = q - qh (fp16 residual)
                    nc.vector.tensor_tensor(nl[:, hs], qf[:, hs], nh[:, hs],
                                            op=ALU.subtract)

            def emit_sends(it):
                """Broadcast my new qh block (fp16, 336 B/partition) to the 3
                peers, landing at span m of the peer's qh buffer. Slots 0-2
                only: bit-2 slots are reserved for cross-die dests and our
                peers are same-die."""
                nh = qh[(it + 1) % 2]
                for m in (1, 2, 3):
                    rdests = [None] * 8
                    rdests[m - 1] = (0, m)
                    nc.gpsimd.remote_dma_broadcast(
                        nh[:, m * PC * C:(m + 1) * PC * C], nh[:, 0:PC * C],
                        rsems[m - 1], lsem, rdests=rdests)
                nc.gpsimd.trigger_dma(count=None)

            def emit_gather(it):
                """qyl rows for round `it` rebuilt from qh[it%2] spans via a
                DRAM bounce. Span 0 ungated; remote spans gated on their
                rsem. Returns the row-gather DMAs (conv deps)."""
                buf = qh[it % 2]
                scr = nc.dram_tensor(f"qscr{it}", [NB * 128, PC * C], FP16,
                                     kind="Internal")
                cp0 = nc.sync.dma_start(scr[0:128, :], buf[:, 0:PC * C])
                prev = cp0
                for s in range(1, NB):
                    w = nc.sync.wait_ge(rsems[s - 1], RSEM_PER_ROUND * it)
                    add_dep_helper(w.ins, prev.ins,
                                   reason="span bounce waits p2p sem")
                    cp = nc.sync.dma_start(
                        scr[s * 128:(s + 1) * 128, :],
                        buf[:, s * PC * C:(s + 1) * PC * C])
                    add_dep_helper(cp.ins, w.ins,
                                   reason="span bounce after p2p wait")
                    prev = cp
                src_y = scr[:].rearrange(
                    "(s ylo x) (pcl c) -> s ylo pcl x c", s=NB, ylo=2, c=C)
                deps = []
                for s in range(NB):
                    dsts = qyl[s * 16:(s + 1) * 16, :].rearrange(
                        "(pcl ylo) (x c) -> ylo pcl x c", ylo=2, c=C)
                    for ylo in range(2):
                        eng = nc.scalar if (2 * s + ylo) % 2 else nc.sync
                        g = eng.dma_start(dsts[ylo], src_y[s, ylo])
                        deps.append(g)
                return deps

            # ================= build phase + iteration 0 =================
            wb = ps_build.tile([128, 512], F32, tag="pbuild")
            for _ in range(12):
                nc.tensor.matmul(wb[:, :128], ident[:], ident[:],
                                 start=True, stop=True)

            # conv for iteration 0 from host qyl0 (ungated)
            pt2 = ps_conv.tile([128, 512], F32, tag="pt2")
            emit_conv([])
            emit_conv_tail()

            # W build (fp16 out) interleaved with iteration-0 qbf: one
            # [128,53] stationary [qh|pad|ql] per tile puts hi at psum rows
            # 0-20 and lo at rows 32-52 in a single 512-col matmul.
            # qbf0 lags the build by TWO tiles so its exp gate is already
            # open when it issues -- a 1-tile lag leaves a sem-wait sliver
            # each tile and the HAM clock-gate halves the PE clock.
            pq0 = ps_qbf.tile([128, 1024], F32, tag="pqbf")

            def mm0(j, h, start, stop):
                return nc.tensor.matmul(
                    pq0[0:53, h * 512:(h + 1) * 512],
                    qhl0i[:, j * 53:(j + 1) * 53],
                    w_sb[:, j * PB + h * 512: j * PB + (h + 1) * 512],
                    start=start, stop=stop)

            for j in range(T):
                for hh in (0, 1):
                    pb = ps_build.tile([128, 512], F32, tag="pbuild")
                    nc.tensor.matmul(
                        pb[:], fa[:, j * 128:(j + 1) * 128],
                        fb[:, hh * 512:(hh + 1) * 512],
                        start=True, stop=True)
                    nc.scalar.activation(
                        w_sb[:, j * PB + hh * 512: j * PB + (hh + 1) * 512],
                        pb[:], EXP, bias=0.0, scale=1.0)
                if j >= 2:
                    jj = j - 2
                    for h in (0, 1):
                        mm0(jj, h, start=(jj == 0), stop=False)
            for jj in (T - 2, T - 1):
                for h in (0, 1):
                    mm0(jj, h, start=False, stop=(jj == T - 1))

            # ================= iterations =================
            pq = pq0
            for it in range(NUM_ITER):
                if it > 0:
                    bh, bl = qh[it % 2], ql[it % 2]
                    pq = ps_qbf.tile([128, 1024], F32, tag="pqbf")
                    # local span (own block, hi+lo) -- before any p2p wait
                    prev = None
                    for j in range(PC):
                        for h in (0, 1):
                            mm_hi(pq, bh, j, h, start=(j == 0), stop=False)
                            prev = mm_lo(pq, bl, j, h, start=(j == 0),
                                         stop=(j == PC - 1))
                    # keep-warm fillers bridge part of the transport wait.
                    # Chained onto the local stream: free-floating fillers get
                    # hoisted into the build phase by the scheduler and drag
                    # the p2p waits with them => cross-core deadlock.
                    for _ in range(2):
                        f = filler(pq)
                        add_dep_helper(f.ins, prev.ins,
                                       reason="filler pinned after local mms")
                        prev = f
                    # remote spans (qh only), gated per span, consumed in
                    # the measured ARRIVAL order of the three messages
                    # (1, 3, 2 -- each sender's ring serializes its three
                    # messages, and span 2's lands last).
                    for s in (1, 3, 2):
                        w = nc.tensor.wait_ge(rsems[s - 1],
                                              RSEM_PER_ROUND * it)
                        add_dep_helper(w.ins, prev.ins,
                                       reason="span wait ordered on queue")
                        prev = w
                        for h in (0, 1):
                            for j in range(s * PC, (s + 1) * PC):
                                mm = mm_hi(pq, bh, j, h, start=False,
                                           stop=(s == 2 and h == 1
                                                 and j == 3 * PC - 1))
                                add_dep_helper(mm.ins, w.ins,
                                               reason="remote qbf waits span")
                                prev = mm
                        if s == 2:
                            prev = emit_conv([w] + gather_deps)
                            emit_conv_tail()

                pbt = ps_conv.tile([128, 512], F32, tag="pt2")
                for h in (0, 1):
                    emit_epilogue(pq, it, h)

                if it < NUM_ITER - 1:
                    emit_sends(it)
                    pt2 = ps_conv.tile([128, 512], F32, tag="pt2")
                    gather_deps = emit_gather(it + 1)
                else:
                    out_dma = nc.sync.dma_start(out_d[:], qout[:])

            # quiesce + clear the p2p semaphores so re-execution starts clean
            wq = out_dma
            for rs in rsems:
                w = nc.sync.wait_ge(rs, RSEM_PER_ROUND * (NUM_ITER - 1))
                add_dep_helper(w.ins, wq.ins, reason="quiesce order")
                wq = w
            wl = nc.sync.wait_ge(lsem, LSEM_PER_ROUND * (NUM_ITER - 1))
            add_dep_helper(wl.ins, wq.ins, reason="quiesce order")
            wq = wl
            for sem in (*rsems, lsem):
                cl = nc.sync.sem_clear(sem)
                add_dep_helper(cl.ins, wq.ins, reason="clear after quiesce")
                wq = cl

    nc.compile()
    return nc


def _host_inputs(unary, ref, gk, kstd):
    """Build the 8 per-core input maps (fp64 host math).

    XOR block layout: core with group-rank g sees pixel-block (g^s) at
    span/tile-group s. fa columns, qh0/ql0 tiles, q0yl row-groups and ay
    rows are permuted accordingly.
    """
    unary = np.asarray(unary, np.float64)
    ref = np.asarray(ref, np.float64)
    gk = np.asarray(gk, np.float64)
    kstd = np.asarray(kstd, np.float64)

    yy, xx = np.meshgrid(np.arange(H, dtype=np.float64),
                         np.arange(W_IMG, dtype=np.float64), indexing="ij")
    grid = np.broadcast_to(np.stack([yy, xx])[None], (N, 2, H, W_IMG))
    stacked = np.concatenate([grid, ref], axis=1)
    feats = (stacked / kstd[None, :, None, None]).reshape(N, 5, P)  # [N,5,P]

    # hi/lo split so every matmul operand is exact(ish) in fp16's 11-bit
    # mantissa; products are exact in the PE multiplier, summed in fp32 PSUM.
    ctr = np.array([31.5 / kstd[0], 31.5 / kstd[1],
                    127.5 / kstd[2], 127.5 / kstd[3], 127.5 / kstd[4]])
    fc = feats - ctr[None, :, None]
    fs = np.round(fc[:, :2] * 8192) / 8192          # spatial, 2^-13 grid
    hh = np.round(fc[:, 2:] * 64) / 64              # color hi, 2^-6 grid
    ll = fc[:, 2:] - hh                             # color lo (|l| <= 2^-7)
    Feff = np.concatenate([fs, hh + ll], axis=1)
    sq = np.sum(Feff * Feff, axis=1)                # [N,P]
    ln4 = np.log(COMPAT_BF)

    U = np.log(np.clip(unary, 1e-5, 1.0)).reshape(N, C, P)
    q0 = np.exp(U - U.max(axis=1, keepdims=True))
    q0 = q0 / q0.sum(axis=1, keepdims=True)

    g2 = gk[0, 0]
    v = g2[:, 35] / np.sqrt(g2[35, 35])
    A = np.zeros((64, 64), np.float64)
    for a in range(64):
        for b in range(64):
            if abs(b - a) <= 35:
                A[a, b] = v[b - a + 35]

    in_maps = []
    for core in range(8):
        n, g = core // NB, core % NB
        blk = slice(g * PB, (g + 1) * PB)
        # pixel permutation: tile-group s covers block g^s
        perm = np.concatenate(
            [np.arange((g ^ s) * PB, (g ^ s) * PB + PB) for s in range(NB)])
        one = np.ones(P)
        Hq = np.round(-0.5 * sq[n] * 8) / 8
        Lq = -0.5 * sq[n] - Hq
        Hp = np.round((-0.5 * sq[n] + ln4) * 8) / 8
        Lp = (-0.5 * sq[n] + ln4) - Hp
        a_dims = [fs[n][0], fs[n][1]]
        b_dims = [fs[n][0], fs[n][1]]
        for ci in range(3):
            a_dims += [hh[n][ci], hh[n][ci], ll[n][ci], ll[n][ci]]
            b_dims += [hh[n][ci], ll[n][ci], hh[n][ci], ll[n][ci]]
        a_dims += [Hq, Lq, one, one]
        b_dims += [one, one, Hp, Lp]
        fa = np.stack(a_dims)[:, perm].astype(np.float16)   # [18, P]
        fb = np.stack(b_dims)[:, blk].astype(np.float16)    # [18, PB]
        u_blk = (U[n].T[blk]
                 .reshape(PC, 128, C).transpose(1, 0, 2)
                 .reshape(128, PC * C).astype(np.float32))
        # q0 in permuted pixel-chunk layout, fp16 hi/lo pair
        q0p = q0[n].T[perm]                                  # [P, C]
        q0pc = (q0p.reshape(T, 128, C).transpose(1, 0, 2)
                .reshape(128, T * C))
        qh0 = q0pc.astype(np.float16)
        ql0 = (q0pc - qh0.astype(np.float64)).astype(np.float16)
        qhl0i = np.zeros((128, T, 53), np.float16)
        qhl0i[:, :, 0:C] = qh0.reshape(128, T, C)
        qhl0i[:, :, 32:53] = ql0.reshape(128, T, C)
        qhl0i = qhl0i.reshape(128, T * 53)
        rowperm = np.concatenate(
            [np.arange((g ^ s) * 16, (g ^ s) * 16 + 16) for s in range(NB)])
        q0yl = (q0[n].T.reshape(H, W_IMG * C)[rowperm].astype(np.float16))
        ay = A[rowperm][:, g * 16:(g + 1) * 16]
        in_maps.append({
            "fa": fa, "fb": fb, "u_blk": u_blk,
            "qhl0i": qhl0i, "q0yl": q0yl,
            "a2mat": (COMPAT_SPATIAL * A).astype(np.float16),
            "ay": ay.astype(np.float16),
        })
    return in_maps


def kernel(unary, ref, gk, kstd):
    global _CACHED_NC, LAST_EXEC_NS, LAST_RESULTS
    in_maps = _host_inputs(unary, ref, gk, kstd)
    if _CACHED_NC is None:
        _CACHED_NC = _build_program()
    res = run_bass_kernel_spmd(_CACHED_NC, in_maps, core_ids=list(range(8)),
                               trace=TRACE)
    LAST_EXEC_NS = res.exec_time_ns
    LAST_RESULTS = res
    q_full = np.zeros((N, P, C), np.float32)
    for core in range(8):
        n, g = core // NB, core % NB
        blk = res.results[core]["out_blk"]
        q_full[n, g * PB:(g + 1) * PB] = (
            blk.reshape(128, PC, C).transpose(1, 0, 2).reshape(PB, C))
    return q_full.transpose(0, 2, 1).reshape(N, C, H, W_IMG).astype(np.float32)
